# revision 1
# baseline (speedup 1.0000x reference)
"""GNN NodeBlock kernel for Trainium2, 8 NeuronCores (SPMD, no collectives).

Reference computation (N=50000 nodes, E=1600000 edges, F=128 features):
    recv_agg = segment_sum(edge_attr, edge_index[1], N)        # [N, 128]
    collected = concat([recv_agg, x, broadcast(u)], -1)        # [N, 272]
    out = relu(collected @ W1 + b1) @ W2 + b2                  # [N, 128]

Sharding: nodes are partitioned into 8 contiguous blocks of 6250; edges are
bucketed by receiver-node ownership on the host so the scatter-sum is local
to each core; MLP weights are replicated.  The u-term of layer 1 is constant
across nodes and is folded into b1 on the host (b1_eff = b1 + u @ W1[256:]).

Device algorithm per core (nodes padded to 196 tiles of 32):
  scatter: edges sorted by receiver, grouped per 64-node tile, padded to
    C_t chunks of 128 edges (C_t = per-tile max over the 8 cores, so the
    program is identical across cores).  One DVE tensor_tensor (is_equal)
    per tile builds all C_t one-hot routing blocks
    onehot[e, c, n] = (rel[e, c] == n) by comparing a broadcast iota row
    against broadcast per-chunk receiver offsets; the PE accumulates
    aggT[f, n] += edge_chunk[e, f]^T @ onehot[e, c, :] into PSUM.  aggT
    comes out feature-major — exactly the layout layer 1 needs.
  L1: out1T[h, n] = W1a[d, h]^T @ aggT[d, n] + W1b[d, h]^T @ xT[d, n] over
    supertiles of up to 512 nodes, then ReLU+bias on the scalar engine.
  L2: out2[n, f] = sum_h hT[h, n]^T @ W2r[h, f]; b2 is added during the
    PSUM evacuation.
All matmul inputs are bf16 (fp32 PSUM accumulation); one-hots are exact.
"""

import numpy as np

from concourse import bacc, mybir, tile
from concourse import bass_utils
from concourse.bass_interp import get_hw_module

# ---------------- problem constants (hardcoded per spec) ----------------
N_NODES = 50000
N_EDGES = 1600000
F = 128           # edge/node feature dim
H = 1024          # hidden dim
D_U = 16
N_CORES = 8
NODES_PC = N_NODES // N_CORES          # 6250
TN = 32                                # nodes per scatter tile
NT = (NODES_PC + TN - 1) // TN         # 196 scatter tiles per core
NODES_PAD = NT * TN                    # 6272
SUP = 16                               # scatter tiles per supertile (512 nodes)
SUPERS = [SUP] * (NT // SUP) + ([NT % SUP] if NT % SUP else [])  # [16]*12+[4]

EDGE_DT = mybir.dt.bfloat16
MLP_DT = mybir.dt.bfloat16
OUT_DT = mybir.dt.bfloat16             # on-device output store dtype

_np = mybir.dt.np  # mybir dtype -> numpy dtype


# ---------------- device program ----------------

def build_program(c_tile, edge_dt, mlp_dt):
    """c_tile: tuple of per-tile chunk counts (len NT), identical across
    cores."""
    f32 = mybir.dt.float32
    NS = len(SUPERS)
    HC = H // 128  # 8 hidden chunks
    c_off = np.zeros(NT + 1, np.int64)
    np.cumsum(np.asarray(c_tile), out=c_off[1:])
    QT = int(c_off[-1])                # total chunks per core
    c_max = int(max(c_tile))

    nc = bacc.Bacc("TRN2", target_bir_lowering=False, debug=False,
                   num_devices=N_CORES)

    edges = nc.dram_tensor("edges", [128, QT, F], edge_dt,
                           kind="ExternalInput").ap()
    relT = nc.dram_tensor("relT", [128, QT], edge_dt,
                          kind="ExternalInput").ap()
    iota = nc.dram_tensor("iota", [128, TN], edge_dt,
                          kind="ExternalInput").ap()
    xT = nc.dram_tensor("xT", [128, NODES_PAD], mlp_dt,
                        kind="ExternalInput").ap()
    w1a = nc.dram_tensor("w1a", [128, H], mlp_dt, kind="ExternalInput").ap()
    w1b = nc.dram_tensor("w1b", [128, H], mlp_dt, kind="ExternalInput").ap()
    w2r = nc.dram_tensor("w2r", [128, H], mlp_dt, kind="ExternalInput").ap()
    b1T = nc.dram_tensor("b1T", [128, HC], f32, kind="ExternalInput").ap()
    b2bc = nc.dram_tensor("b2bc", [128, F], f32, kind="ExternalInput").ap()
    y = nc.dram_tensor("y", [NODES_PAD, F], OUT_DT,
                       kind="ExternalOutput").ap()
    # [p, g, f] view of y for batched stores (g = 128-node block)
    yr = y.rearrange("(g p) f -> p g f", p=128)

    with tile.TileContext(nc) as tc:
        with (
            tc.tile_pool(name="const", bufs=1) as cpool,
            tc.tile_pool(name="edge", bufs=2) as epool,
            tc.tile_pool(name="oh", bufs=4) as ohpool,
            tc.tile_pool(name="agg", bufs=2) as aggpool,
            tc.tile_pool(name="h", bufs=2) as hpool,
            tc.tile_pool(name="out", bufs=2) as outpool,
            tc.tile_pool(name="ps_agg", bufs=2, space="PSUM") as ps_agg,
            tc.tile_pool(name="ps_h", bufs=2, space="PSUM") as ps_h,
            tc.tile_pool(name="ps_out", bufs=2, space="PSUM") as ps_out,
        ):
            # resident constants — loaded on the scalar HWDGE ring so the
            # edge stream (sync ring) starts immediately
            relT_sb = cpool.tile([128, QT], edge_dt, tag="relT")
            nc.scalar.dma_start(relT_sb[:], relT[:])
            iota_sb = cpool.tile([128, TN], edge_dt, tag="iota")
            nc.scalar.dma_start(iota_sb[:], iota[:])
            xT_sb = cpool.tile([128, NODES_PAD], mlp_dt, tag="xT")
            nc.scalar.dma_start(xT_sb[:], xT[:])
            w1a_sb = cpool.tile([128, H], mlp_dt, tag="w1a")
            nc.scalar.dma_start(w1a_sb[:], w1a[:])
            w1b_sb = cpool.tile([128, H], mlp_dt, tag="w1b")
            nc.scalar.dma_start(w1b_sb[:], w1b[:])
            w2r_sb = cpool.tile([128, H], mlp_dt, tag="w2r")
            nc.scalar.dma_start(w2r_sb[:], w2r[:])
            b1T_sb = cpool.tile([128, HC], f32, tag="b1T")
            nc.scalar.dma_start(b1T_sb[:], b1T[:])
            b2bc_sb = cpool.tile([128, F], f32, tag="b2bc")
            nc.scalar.dma_start(b2bc_sb[:], b2bc[:])

            t0 = 0
            for s, nts in enumerate(SUPERS):
                nn = nts * TN
                q0, q1 = int(c_off[t0]), int(c_off[t0 + nts])
                e_sup = epool.tile([128, q1 - q0, F], edge_dt, tag="e")
                nc.sync.dma_start(e_sup[:], edges[:, q0:q1])
                aggT = aggpool.tile([128, nn], mlp_dt, tag="aggT")
                for st in range(nts):
                    t = t0 + st
                    ct = int(c_tile[t])
                    tq0 = int(c_off[t]) - q0
                    # one-hot blocks for all chunks of this tile in one op
                    oh = ohpool.tile([128, ct, TN], edge_dt, tag="oh")
                    rel_bc = relT_sb[:, tq0 + q0:tq0 + q0 + ct].rearrange(
                        "p (c u) -> p c u", u=1).broadcast_to([128, ct, TN])
                    iota_bc = iota_sb[:].rearrange(
                        "p (u n) -> p u n", u=1).broadcast_to([128, ct, TN])
                    nc.vector.tensor_tensor(out=oh[:], in0=iota_bc,
                                            in1=rel_bc,
                                            op=mybir.AluOpType.is_equal)
                    agg_ps = ps_agg.tile([128, TN], f32, tag="agg")
                    for c in range(ct):
                        nc.tensor.matmul(
                            agg_ps[:],
                            lhsT=e_sup[:, tq0 + c, :],
                            rhs=oh[:, c, :],
                            start=(c == 0),
                            stop=(c == ct - 1),
                        )
                    # evacuate psum, alternating engines (casts to bf16)
                    if st % 2 == 0:
                        nc.scalar.copy(aggT[:, st * TN:(st + 1) * TN],
                                       agg_ps[:])
                    else:
                        nc.vector.tensor_copy(aggT[:, st * TN:(st + 1) * TN],
                                              agg_ps[:])
                # layer 1 over the supertile, hidden chunk by hidden chunk
                hT = hpool.tile([128, HC, nn], mlp_dt, tag="hT")
                for hc in range(HC):
                    h_ps = ps_h.tile([128, nn], f32, tag="h")
                    nc.tensor.matmul(h_ps[:],
                                     lhsT=w1a_sb[:, hc * 128:(hc + 1) * 128],
                                     rhs=aggT[:],
                                     start=True, stop=False)
                    nc.tensor.matmul(h_ps[:],
                                     lhsT=w1b_sb[:, hc * 128:(hc + 1) * 128],
                                     rhs=xT_sb[:, t0 * TN:t0 * TN + nn],
                                     start=False, stop=True)
                    nc.scalar.activation(hT[:, hc, :], h_ps[:],
                                         mybir.ActivationFunctionType.Relu,
                                         bias=b1T_sb[:, hc:hc + 1], scale=1.0)
                # layer 2 per 128-node block
                ng = nn // 128
                o_sup = outpool.tile([128, ng, F], OUT_DT, tag="o")
                for g in range(ng):
                    o_ps = ps_out.tile([128, F], f32, tag="ops")
                    for hc in range(HC):
                        nc.tensor.matmul(
                            o_ps[:],
                            lhsT=hT[:, hc, g * 128:(g + 1) * 128],
                            rhs=w2r_sb[:, hc * 128:(hc + 1) * 128],
                            start=(hc == 0), stop=(hc == HC - 1))
                    # evacuate psum + add b2 in one pass
                    nc.vector.tensor_tensor(out=o_sup[:, g, :], in0=o_ps[:],
                                            in1=b2bc_sb[:],
                                            op=mybir.AluOpType.add)
                g0 = t0 * TN // 128
                nc.scalar.dma_start(yr[:, g0:g0 + ng, :], o_sup[:])
                t0 += nts

    nc.compile()
    nc.m = get_hw_module(nc.m)
    return nc


# ---------------- host-side sharding / packing ----------------

def prepare_inputs(x, edge_attr, u, W1, b1, W2, b2, edge_index):
    x = np.asarray(x, dtype=np.float32)
    edge_attr = np.asarray(edge_attr, dtype=np.float32)
    u = np.asarray(u, dtype=np.float32)
    W1 = np.asarray(W1, dtype=np.float32)
    b1 = np.asarray(b1, dtype=np.float32)
    W2 = np.asarray(W2, dtype=np.float32)
    b2 = np.asarray(b2, dtype=np.float32)
    recv = np.asarray(edge_index)[1].astype(np.int64)

    edge_np = _np(EDGE_DT)
    mlp_np = _np(MLP_DT)

    # sort edges by receiver; contiguous receiver ranges per core
    order = np.argsort(recv, kind="stable")
    recv_s = recv[order]
    bounds = np.searchsorted(recv_s, np.arange(0, N_NODES + 1, NODES_PC))

    # per-(core,tile) edge counts -> per-tile chunk capacity (max over cores)
    per_core = []
    cnts = np.zeros((N_CORES, NT), np.int64)
    for c in range(N_CORES):
        sl = slice(bounds[c], bounds[c + 1])
        loc = recv_s[sl] - c * NODES_PC
        tid = loc // TN
        cnt = np.bincount(tid, minlength=NT)
        cnts[c] = cnt
        per_core.append((order[sl], loc, tid, cnt))
    c_tile = tuple(int(v) for v in -(-cnts.max(axis=0) // 128))
    c_off = np.zeros(NT + 1, np.int64)
    np.cumsum(np.asarray(c_tile), out=c_off[1:])
    QT = int(c_off[-1])

    # shared (replicated) tensors
    b1_eff = b1 + (u[0] @ W1[256:256 + D_U])
    w1a = np.ascontiguousarray(W1[0:128]).astype(mlp_np)
    w1b = np.ascontiguousarray(W1[128:256]).astype(mlp_np)
    w2r = np.ascontiguousarray(
        W2.reshape(H // 128, 128, F).transpose(1, 0, 2).reshape(128, H)
    ).astype(mlp_np)
    b1T = np.ascontiguousarray(
        b1_eff.reshape(H // 128, 128).T).astype(np.float32)
    b2bc = np.ascontiguousarray(np.tile(b2, (128, 1))).astype(np.float32)
    iota = np.tile(np.arange(TN, dtype=np.float32), (128, 1)).astype(edge_np)

    in_maps = []
    for c in range(N_CORES):
        perm, loc, tid, cnt = per_core[c]
        off = np.zeros(NT, np.int64)
        np.cumsum(cnt[:-1], out=off[1:])
        s = np.arange(len(perm), dtype=np.int64) - off[tid]   # rank in tile
        p = s & 127
        ch = s >> 7
        q = c_off[tid] + ch                                   # global chunk
        slot = p * QT + q

        ebuf = np.zeros((128 * QT, F), edge_np)
        ebuf[slot] = edge_attr[perm].astype(edge_np)
        ebuf = ebuf.reshape(128, QT, F)

        rel = np.full(128 * QT, -1.0, np.float32)
        rel[slot] = (loc - tid * TN).astype(np.float32)
        relT = rel.reshape(128, QT).astype(edge_np)

        xs = x[c * NODES_PC:(c + 1) * NODES_PC]
        xT = np.zeros((128, NODES_PAD), mlp_np)
        xT[:, :NODES_PC] = xs.T.astype(mlp_np)

        in_maps.append({
            "edges": ebuf, "relT": relT, "iota": iota, "xT": xT,
            "w1a": w1a, "w1b": w1b, "w2r": w2r, "b1T": b1T, "b2bc": b2bc,
        })
    return in_maps, c_tile


_prog_cache = {}


def _get_program(c_tile):
    key = (c_tile, EDGE_DT, MLP_DT, OUT_DT)
    if key not in _prog_cache:
        _prog_cache[key] = build_program(c_tile, EDGE_DT, MLP_DT)
    return _prog_cache[key]


def run(inputs, trace=False, tmpdir=None):
    in_maps, c_tile = prepare_inputs(**inputs)
    nc = _get_program(c_tile)
    res = bass_utils.run_bass_kernel_spmd(
        nc, in_maps, core_ids=list(range(N_CORES)), trace=trace,
        tmpdir=tmpdir)
    out = np.concatenate(
        [np.asarray(res.results[c]["y"][:NODES_PC], dtype=np.float32)
         for c in range(N_CORES)], axis=0)
    return out, res


def kernel(**inputs) -> np.ndarray:
    out, _ = run(inputs, trace=False)
    return out



# revision 2
# speedup vs baseline: 1.7268x; 1.7268x over previous
"""GNN NodeBlock kernel for Trainium2, 8 NeuronCores (SPMD, no collectives).

Reference computation (N=50000 nodes, E=1600000 edges, F=128 features):
    recv_agg = segment_sum(edge_attr, edge_index[1], N)        # [N, 128]
    collected = concat([recv_agg, x, broadcast(u)], -1)        # [N, 272]
    out = relu(collected @ W1 + b1) @ W2 + b2                  # [N, 128]

Host-side sharding: nodes are re-balanced into 1584 bins of <=32 nodes
with <=1024 edges each (snake assignment of degree-sorted nodes), so
every bin needs exactly 8 edge-chunks of 128 — a fully static, uniform
program across all 8 cores (198 bins/core, 1584 chunks/core vs ~1758
for contiguous sharding).  Edges are packed fp8(e4m3) with
error-feedback quantization per receiver run: the device's fp32 PSUM
sum of the quantized stream telescopes to the true sum minus one final
sub-ulp carry (rel err ~6e-3 vs 2.9e-2 for plain fp8 rounding).  The
u-term of layer 1 is folded into b1 on the host.

Device program per core, one supertile (16 bins = 512 nodes) at a time:
  scatter: one DVE is_equal builds all 128 one-hot blocks
    onehot[e, q, n] = (rel[e, q] == n) in fp8; the PE accumulates
    aggT[f, 32-node window] += chunk^T @ onehot into a [128, 512] PSUM
    bank (8 chunks per bin, start/stop per bin).  fp8 weights get FWL
    (4 elem/cycle LDWEIGHTS), so the 32-wide matmuls run at the ~60
    cycle dispatch floor.
  L1: h_ps[h, n] = W1a^T @ aggT + W1b^T @ xT per 128-hidden chunk;
    ReLU+bias evacuation alternates scalar/vector engines.
  L2: o_ps[f, n] = sum_hc W2r_hc^T @ hT_hc (weights stationary,
    feature-major output); bias b2 added during PSUM evacuation; host
    transposes the output back to node-major.
Everything streams behind the PE: the edge DMA (2 MB fp8/supertile) and
all DVE/ACT work fit well under the ~8.4 us/supertile of matmul.
"""

import numpy as np

from concourse import bacc, mybir, tile
from concourse import bass_utils
from concourse.bass_interp import get_hw_module

# ---------------- problem constants (hardcoded per spec) ----------------
N_NODES = 50000
N_EDGES = 1600000
F = 128           # edge/node feature dim
H = 1024          # hidden dim
HC = H // 128     # 8 hidden chunks
D_U = 16
N_CORES = 8
TN = 32                                # nodes per bin (one-hot window)
NT = 198                               # bins per core
B_GLOBAL = N_CORES * NT                # 1584 bins
CPB = 8                                # chunks per bin (bin cap = 1024 edges)
QT = NT * CPB                          # 1584 chunks per core
NODES_PAD = NT * TN                    # 6336 node slots per core
SUP = 16                               # bins per supertile (512 nodes)
SUPERS = [SUP] * (NT // SUP) + ([NT % SUP] if NT % SUP else [])  # [16]*12+[6]

EDGE_DT = mybir.dt.float8e4            # ml_dtypes.float8_e4m3
IDX_DT = mybir.dt.bfloat16             # rel codes / iota (ints 0..31 exact)
MLP_DT = mybir.dt.bfloat16
OUT_DT = mybir.dt.bfloat16             # on-device output store dtype

_np = mybir.dt.np  # mybir dtype -> numpy dtype


# ---------------- device program ----------------

def build_program():
    f32 = mybir.dt.float32

    nc = bacc.Bacc("TRN2", target_bir_lowering=False, debug=False,
                   num_devices=N_CORES)

    edges = nc.dram_tensor("edges", [128, QT, F], EDGE_DT,
                           kind="ExternalInput").ap()
    relT = nc.dram_tensor("relT", [128, QT], IDX_DT,
                          kind="ExternalInput").ap()
    iota = nc.dram_tensor("iota", [128, TN], IDX_DT,
                          kind="ExternalInput").ap()
    xT = nc.dram_tensor("xT", [128, NODES_PAD], MLP_DT,
                        kind="ExternalInput").ap()
    w1a = nc.dram_tensor("w1a", [128, H], MLP_DT, kind="ExternalInput").ap()
    w1b = nc.dram_tensor("w1b", [128, H], MLP_DT, kind="ExternalInput").ap()
    w2r = nc.dram_tensor("w2r", [128, H], MLP_DT, kind="ExternalInput").ap()
    b1T = nc.dram_tensor("b1T", [128, HC], f32, kind="ExternalInput").ap()
    b2T = nc.dram_tensor("b2T", [128, 1], f32, kind="ExternalInput").ap()
    y = nc.dram_tensor("y", [128, NODES_PAD], OUT_DT,
                       kind="ExternalOutput").ap()

    with tile.TileContext(nc) as tc:
        with (
            tc.tile_pool(name="const", bufs=1) as cpool,
            tc.tile_pool(name="edge", bufs=2) as epool,
            tc.tile_pool(name="oh", bufs=2) as ohpool,
            tc.tile_pool(name="agg", bufs=2) as aggpool,
            tc.tile_pool(name="h", bufs=2) as hpool,
            tc.tile_pool(name="out", bufs=2) as outpool,
            tc.tile_pool(name="ps_agg", bufs=2, space="PSUM") as ps_agg,
            tc.tile_pool(name="ps_h", bufs=2, space="PSUM") as ps_h,
            tc.tile_pool(name="ps_out", bufs=2, space="PSUM") as ps_out,
        ):
            # resident constants on the scalar HWDGE ring; the edge
            # stream (sync ring) starts concurrently
            relT_sb = cpool.tile([128, QT], IDX_DT, tag="relT")
            nc.scalar.dma_start(relT_sb[:], relT[:])
            iota_sb = cpool.tile([128, TN], IDX_DT, tag="iota")
            nc.scalar.dma_start(iota_sb[:], iota[:])
            w1a_sb = cpool.tile([128, H], MLP_DT, tag="w1a")
            nc.scalar.dma_start(w1a_sb[:], w1a[:])
            w1b_sb = cpool.tile([128, H], MLP_DT, tag="w1b")
            nc.scalar.dma_start(w1b_sb[:], w1b[:])
            w2r_sb = cpool.tile([128, H], MLP_DT, tag="w2r")
            nc.scalar.dma_start(w2r_sb[:], w2r[:])
            b1T_sb = cpool.tile([128, HC], f32, tag="b1T")
            nc.scalar.dma_start(b1T_sb[:], b1T[:])
            b2T_sb = cpool.tile([128, 1], f32, tag="b2T")
            nc.scalar.dma_start(b2T_sb[:], b2T[:])
            xT_sb = cpool.tile([128, NODES_PAD], MLP_DT, tag="xT")
            nc.scalar.dma_start(xT_sb[:], xT[:])

            t0 = 0
            for s, nts in enumerate(SUPERS):
                nn = nts * TN
                nq = nts * CPB
                q0 = t0 * CPB
                n0 = t0 * TN
                e_sup = epool.tile([128, nq, F], EDGE_DT, tag="e")
                nc.sync.dma_start(e_sup[:], edges[:, q0:q0 + nq])
                # all one-hot blocks of the supertile in one DVE op
                oh = ohpool.tile([128, nq, TN], EDGE_DT, tag="oh")
                rel_bc = relT_sb[:, q0:q0 + nq].rearrange(
                    "p (c u) -> p c u", u=1).broadcast_to([128, nq, TN])
                iota_bc = iota_sb[:].rearrange(
                    "p (u n) -> p u n", u=1).broadcast_to([128, nq, TN])
                nc.vector.tensor_tensor(out=oh[:], in0=iota_bc, in1=rel_bc,
                                        op=mybir.AluOpType.is_equal)
                # scatter-sum into one PSUM bank, 32-col window per bin
                agg_ps = ps_agg.tile([128, nn], f32, tag="agg")
                for st in range(nts):
                    for c in range(CPB):
                        q = st * CPB + c
                        nc.tensor.matmul(
                            agg_ps[:, st * TN:(st + 1) * TN],
                            lhsT=e_sup[:, q, :],
                            rhs=oh[:, q, :],
                            start=(c == 0),
                            stop=(c == CPB - 1),
                        )
                aggT = aggpool.tile([128, nn], MLP_DT, tag="aggT")
                nc.vector.tensor_copy(aggT[:], agg_ps[:])
                # layer 1, hidden chunk by hidden chunk
                hT = hpool.tile([128, HC, nn], MLP_DT, tag="hT")
                for hc in range(HC):
                    h_ps = ps_h.tile([128, nn], f32, tag="h")
                    nc.tensor.matmul(h_ps[:],
                                     lhsT=w1a_sb[:, hc * 128:(hc + 1) * 128],
                                     rhs=aggT[:],
                                     start=True, stop=False)
                    nc.tensor.matmul(h_ps[:],
                                     lhsT=w1b_sb[:, hc * 128:(hc + 1) * 128],
                                     rhs=xT_sb[:, n0:n0 + nn],
                                     start=False, stop=True)
                    if hc % 2 == 0:
                        nc.scalar.activation(
                            hT[:, hc, :], h_ps[:],
                            mybir.ActivationFunctionType.Relu,
                            bias=b1T_sb[:, hc:hc + 1], scale=1.0)
                    else:
                        nc.vector.tensor_scalar(
                            out=hT[:, hc, :], in0=h_ps[:],
                            scalar1=b1T_sb[:, hc:hc + 1], scalar2=0.0,
                            op0=mybir.AluOpType.add,
                            op1=mybir.AluOpType.max)
                # layer 2: weights stationary, feature-major output
                o_ps = ps_out.tile([128, nn], f32, tag="ops")
                for hc in range(HC):
                    nc.tensor.matmul(
                        o_ps[:],
                        lhsT=w2r_sb[:, hc * 128:(hc + 1) * 128],
                        rhs=hT[:, hc, :],
                        start=(hc == 0), stop=(hc == HC - 1))
                o_sb = outpool.tile([128, nn], OUT_DT, tag="o")
                nc.scalar.activation(o_sb[:], o_ps[:],
                                     mybir.ActivationFunctionType.Identity,
                                     bias=b2T_sb[:, 0:1], scale=1.0)
                nc.scalar.dma_start(y[:, n0:n0 + nn], o_sb[:])
                t0 += nts

    nc.compile()
    nc.m = get_hw_module(nc.m)
    return nc


# ---------------- host-side sharding / packing ----------------

def _pack_bins(deg):
    """Snake-assign degree-sorted nodes into B_GLOBAL bins of <=32 nodes
    and (statistically) <=1024 edges.  Returns bin id + position-in-bin
    per node."""
    order = np.argsort(-deg, kind="stable")
    B = B_GLOBAL
    bsum = np.zeros(B, np.int64)
    bn = np.zeros(B, np.int64)
    bin_of = np.empty(N_NODES, np.int64)
    pos_of = np.empty(N_NODES, np.int64)
    n = len(order)
    for r in range((n + B - 1) // B):
        take = order[r * B:(r + 1) * B]
        ob = np.argsort(bsum, kind="stable")[:len(take)]
        bin_of[take] = ob
        pos_of[take] = bn[ob]
        bn[ob] += 1
        bsum[ob] += deg[take]
    # safety: if any bin exceeds the 1024-edge cap, move its smallest-
    # degree nodes to the emptiest bins with node room
    while True:
        over = np.flatnonzero(bsum > CPB * 128)
        if not len(over):
            break
        b = over[0]
        nodes = np.flatnonzero(bin_of == b)
        v = nodes[np.argmin(deg[nodes])]
        cand = np.flatnonzero(bn < TN)
        tgt = cand[np.argmin(bsum[cand])]
        if bsum[tgt] + deg[v] > CPB * 128:
            raise RuntimeError("bin packing failed")
        # re-compact positions in source bin
        pos_of[nodes[pos_of[nodes] > pos_of[v]]] -= 1
        bin_of[v] = tgt
        pos_of[v] = bn[tgt]
        bn[tgt] += 1
        bn[b] -= 1
        bsum[tgt] += deg[v]
        bsum[b] -= deg[v]
    return bin_of, pos_of


def prepare_inputs(x, edge_attr, u, W1, b1, W2, b2, edge_index):
    x = np.asarray(x, dtype=np.float32)
    edge_attr = np.asarray(edge_attr, dtype=np.float32)
    u = np.asarray(u, dtype=np.float32)
    W1 = np.asarray(W1, dtype=np.float32)
    b1 = np.asarray(b1, dtype=np.float32)
    W2 = np.asarray(W2, dtype=np.float32)
    b2 = np.asarray(b2, dtype=np.float32)
    recv = np.asarray(edge_index)[1].astype(np.int64)

    edge_np = _np(EDGE_DT)
    idx_np = _np(IDX_DT)
    mlp_np = _np(MLP_DT)

    deg = np.bincount(recv, minlength=N_NODES)
    bin_of, pos_of = _pack_bins(deg)

    # sort edges by (bin, pos-in-bin) of their receiver -> per-node runs
    node_key = bin_of * TN + pos_of            # globally unique per node
    ekey = node_key[recv]
    order = np.argsort(ekey, kind="stable")
    ekey_s = ekey[order]
    ea_s = edge_attr[order]

    # error-feedback fp8 quantization per receiver run: the device's
    # fp32 sum of q equals the true fp32 sum minus the final carry
    run_start = np.r_[True, ekey_s[1:] != ekey_s[:-1]]
    starts = np.flatnonzero(run_start)
    run_id = np.cumsum(run_start) - 1
    pos_in_run = np.arange(len(ekey_s)) - starts[run_id]
    q_s = np.empty((len(ekey_s), F), edge_np)
    carry = np.zeros((len(starts), F), np.float32)
    for p in range(int(pos_in_run.max()) + 1):
        sel = np.flatnonzero(pos_in_run == p)
        if not len(sel):
            break
        r = run_id[sel]
        v = ea_s[sel] + carry[r]
        qv = v.astype(edge_np)
        carry[r] = v - qv.astype(np.float32)
        q_s[sel] = qv

    # slot within bin: rank of edge inside its bin
    ebin = ekey_s // TN
    bin_start = np.searchsorted(ebin, np.arange(B_GLOBAL))
    rank = np.arange(len(ebin)) - bin_start[ebin]
    # chunk within core: bin (local) * CPB + rank // 128
    core_of = ebin // NT
    q_local = (ebin % NT) * CPB + (rank >> 7)
    p_slot = rank & 127
    flat = p_slot * QT + q_local               # within-core flat slot

    # shared (replicated) tensors
    b1_eff = b1 + (u[0] @ W1[256:256 + D_U])
    w1a = np.ascontiguousarray(W1[0:128]).astype(mlp_np)
    w1b = np.ascontiguousarray(W1[128:256]).astype(mlp_np)
    w2r = np.ascontiguousarray(
        W2.reshape(HC, 128, F).transpose(1, 0, 2).reshape(128, H)
    ).astype(mlp_np)
    b1T = np.ascontiguousarray(b1_eff.reshape(HC, 128).T).astype(np.float32)
    b2T = b2.reshape(128, 1).astype(np.float32)
    iota_arr = np.tile(np.arange(TN, dtype=np.float32),
                       (128, 1)).astype(idx_np)

    in_maps = []
    node_slot = np.empty(N_NODES, np.int64)  # per-core slot of each node
    for c in range(N_CORES):
        esel = core_of == c
        ebuf = np.zeros((128 * QT, F), edge_np)
        ebuf[flat[esel]] = q_s[esel]
        rel = np.full(128 * QT, -1.0, np.float32)
        rel[flat[esel]] = (ekey_s[esel] % TN).astype(np.float32)

        nsel = np.flatnonzero((bin_of >= c * NT) & (bin_of < (c + 1) * NT))
        slots = (bin_of[nsel] - c * NT) * TN + pos_of[nsel]
        node_slot[nsel] = slots
        xT_arr = np.zeros((128, NODES_PAD), mlp_np)
        xT_arr[:, slots] = x[nsel].T.astype(mlp_np)

        in_maps.append({
            "edges": ebuf.reshape(128, QT, F),
            "relT": rel.reshape(128, QT).astype(idx_np),
            "iota": iota_arr, "xT": xT_arr,
            "w1a": w1a, "w1b": w1b, "w2r": w2r, "b1T": b1T, "b2T": b2T,
        })
    core_of_node = bin_of // NT
    return in_maps, core_of_node, node_slot


_prog_cache = {}


def _get_program():
    key = (EDGE_DT, MLP_DT, OUT_DT)
    if key not in _prog_cache:
        _prog_cache[key] = build_program()
    return _prog_cache[key]


def run(inputs, trace=False, tmpdir=None):
    in_maps, core_of_node, node_slot = prepare_inputs(**inputs)
    nc = _get_program()
    res = bass_utils.run_bass_kernel_spmd(
        nc, in_maps, core_ids=list(range(N_CORES)), trace=trace,
        tmpdir=tmpdir)
    out = np.empty((N_NODES, F), np.float32)
    for c in range(N_CORES):
        yc = np.asarray(res.results[c]["y"], dtype=np.float32)  # [128, PAD]
        nsel = np.flatnonzero(core_of_node == c)
        out[nsel] = yc[:, node_slot[nsel]].T
    return out, res


def kernel(**inputs) -> np.ndarray:
    out, _ = run(inputs, trace=False)
    return out


# revision 4
# speedup vs baseline: 1.7518x; 1.0145x over previous
"""GNN NodeBlock kernel for Trainium2, 8 NeuronCores (SPMD, no collectives).

Reference computation (N=50000 nodes, E=1600000 edges, F=128 features):
    recv_agg = segment_sum(edge_attr, edge_index[1], N)        # [N, 128]
    collected = concat([recv_agg, x, broadcast(u)], -1)        # [N, 272]
    out = relu(collected @ W1 + b1) @ W2 + b2                  # [N, 128]

Host-side sharding: nodes are re-balanced into 1584 bins of <=32 nodes
with <=1024 edges each (snake assignment of degree-sorted nodes), so
every bin needs exactly 8 edge-chunks of 128 — a fully static, uniform
program across all 8 cores (198 bins/core, 1584 chunks/core vs ~1758
for contiguous sharding).  Edges are packed fp8(e4m3) with
error-feedback quantization per receiver run: the device's fp32 PSUM
sum of the quantized stream telescopes to the true sum minus one final
sub-ulp carry (rel err ~6e-3 vs 2.9e-2 for plain fp8 rounding).  The
u-term of layer 1 is folded into b1 on the host.

Device program per core, one supertile (16 bins = 512 nodes) at a time:
  scatter: one DVE is_equal builds all 128 one-hot blocks
    onehot[e, q, n] = (rel[e, q] == n) in fp8; the PE accumulates
    aggT[f, 32-node window] += chunk^T @ onehot into a [128, 512] PSUM
    bank (8 chunks per bin, start/stop per bin).  fp8 weights get FWL
    (4 elem/cycle LDWEIGHTS), so the 32-wide matmuls run at the ~60
    cycle dispatch floor.
  L1: h_ps[h, n] = W1a^T @ aggT + W1b^T @ xT per 128-hidden chunk;
    ReLU+bias evacuation alternates scalar/vector engines.
  L2: o_ps[f, n] = sum_hc W2r_hc^T @ hT_hc (weights stationary,
    feature-major output); bias b2 added during PSUM evacuation; host
    transposes the output back to node-major.
Everything streams behind the PE: the edge DMA (2 MB fp8/supertile) and
all DVE/ACT work fit well under the ~8.4 us/supertile of matmul.
"""

import numpy as np

from concourse import bacc, mybir, tile
from concourse import bass_utils
from concourse.bass_interp import get_hw_module

# ---------------- problem constants (hardcoded per spec) ----------------
N_NODES = 50000
N_EDGES = 1600000
F = 128           # edge/node feature dim
H = 1024          # hidden dim
HC = H // 128     # 8 hidden chunks
D_U = 16
N_CORES = 8
TN = 32                                # nodes per bin (one-hot window)
NT = 198                               # bins per core
B_GLOBAL = N_CORES * NT                # 1584 bins
CPB = 8                                # chunks per bin (bin cap = 1024 edges)
QT = NT * CPB                          # 1584 chunks per core
NODES_PAD = NT * TN                    # 6336 node slots per core
SUP = 16                               # bins per supertile (512 nodes)
# first supertiles are small so the first edge slab lands fast and the
# PE starts ~3us in instead of ~16us
SUPERS = [4, 12] + [SUP] * 11 + [NT - 16 - SUP * 11]  # [4,12,16*11,6]
assert sum(SUPERS) == NT

EDGE_DT = mybir.dt.float8e4            # ml_dtypes.float8_e4m3
IDX_DT = mybir.dt.bfloat16             # rel codes / iota (ints 0..31 exact)
MLP_DT = mybir.dt.bfloat16
OUT_DT = mybir.dt.bfloat16             # on-device output store dtype

_np = mybir.dt.np  # mybir dtype -> numpy dtype


# ---------------- device program ----------------

def build_program():
    f32 = mybir.dt.float32

    nc = bacc.Bacc("TRN2", target_bir_lowering=False, debug=False,
                   num_devices=N_CORES)

    edges = nc.dram_tensor("edges", [128, QT, F], EDGE_DT,
                           kind="ExternalInput").ap()
    relT = nc.dram_tensor("relT", [128, QT], IDX_DT,
                          kind="ExternalInput").ap()
    iota = nc.dram_tensor("iota", [128, TN], IDX_DT,
                          kind="ExternalInput").ap()
    xT = nc.dram_tensor("xT", [128, NODES_PAD], MLP_DT,
                        kind="ExternalInput").ap()
    w1a = nc.dram_tensor("w1a", [128, H], MLP_DT, kind="ExternalInput").ap()
    w1b = nc.dram_tensor("w1b", [128, H], MLP_DT, kind="ExternalInput").ap()
    w2r = nc.dram_tensor("w2r", [128, H], MLP_DT, kind="ExternalInput").ap()
    b1T = nc.dram_tensor("b1T", [128, HC], f32, kind="ExternalInput").ap()
    b2T = nc.dram_tensor("b2T", [128, 1], f32, kind="ExternalInput").ap()
    y = nc.dram_tensor("y", [128, NODES_PAD], OUT_DT,
                       kind="ExternalOutput").ap()

    with tile.TileContext(nc) as tc:
        with (
            tc.tile_pool(name="const", bufs=1) as cpool,
            tc.tile_pool(name="edge", bufs=3) as epool,
            tc.tile_pool(name="oh", bufs=3) as ohpool,
            tc.tile_pool(name="agg", bufs=2) as aggpool,
            tc.tile_pool(name="h", bufs=2) as hpool,
            tc.tile_pool(name="out", bufs=2) as outpool,
            tc.tile_pool(name="ps_agg", bufs=2, space="PSUM") as ps_agg,
            tc.tile_pool(name="ps_h", bufs=4, space="PSUM") as ps_h,
            tc.tile_pool(name="ps_out", bufs=2, space="PSUM") as ps_out,
        ):
            # rel codes + iota lead the sync ring (the one-hot for the
            # first supertile needs them); edge slabs follow.  MLP
            # weights stream on the scalar ring concurrently, x head
            # first so L1 of the small first supertiles isn't gated on
            # the full 1.6 MB xT load.
            relT_sb = cpool.tile([128, QT], IDX_DT, tag="relT")
            nc.sync.dma_start(relT_sb[:], relT[:])
            iota_sb = cpool.tile([128, TN], IDX_DT, tag="iota")
            nc.sync.dma_start(iota_sb[:], iota[:])
            w1a_sb = cpool.tile([128, H], MLP_DT, tag="w1a")
            nc.scalar.dma_start(w1a_sb[:], w1a[:])
            w1b_sb = cpool.tile([128, H], MLP_DT, tag="w1b")
            nc.scalar.dma_start(w1b_sb[:], w1b[:])
            XH = 512  # covers the first two supertiles (16 bins)
            xT_sb = cpool.tile([128, NODES_PAD], MLP_DT, tag="xT")
            nc.scalar.dma_start(xT_sb[:, :XH], xT[:, :XH])
            b1T_sb = cpool.tile([128, HC], f32, tag="b1T")
            nc.scalar.dma_start(b1T_sb[:], b1T[:])
            nc.scalar.dma_start(xT_sb[:, XH:], xT[:, XH:])
            w2r_sb = cpool.tile([128, H], MLP_DT, tag="w2r")
            nc.scalar.dma_start(w2r_sb[:], w2r[:])
            b2T_sb = cpool.tile([128, 1], f32, tag="b2T")
            nc.scalar.dma_start(b2T_sb[:], b2T[:])

            iota_bc1 = iota_sb[:].rearrange("p (u n) -> p u n", u=1)

            def make_onehot(s):
                nts_ = SUPERS[s]
                nq_ = nts_ * CPB
                q0_ = sum(SUPERS[:s]) * CPB
                oh_ = ohpool.tile([128, nq_, TN], EDGE_DT, tag="oh")
                rel_bc = relT_sb[:, q0_:q0_ + nq_].rearrange(
                    "p (c u) -> p c u", u=1).broadcast_to([128, nq_, TN])
                nc.vector.tensor_tensor(
                    out=oh_[:], in0=iota_bc1.broadcast_to([128, nq_, TN]),
                    in1=rel_bc, op=mybir.AluOpType.is_equal)
                return oh_

            ohs = {0: make_onehot(0)}
            t0 = 0
            for s, nts in enumerate(SUPERS):
                nn = nts * TN
                nq = nts * CPB
                q0 = t0 * CPB
                n0 = t0 * TN
                e_sup = epool.tile([128, nq, F], EDGE_DT, tag="e")
                nc.sync.dma_start(e_sup[:], edges[:, q0:q0 + nq])
                # build next supertile's one-hots on DVE while the PE
                # scatters this one
                oh = ohs.pop(s)
                if s + 1 < len(SUPERS):
                    ohs[s + 1] = make_onehot(s + 1)
                # scatter-sum into one PSUM bank, 32-col window per bin
                agg_ps = ps_agg.tile([128, nn], f32, tag="agg")
                for st in range(nts):
                    for c in range(CPB):
                        q = st * CPB + c
                        nc.tensor.matmul(
                            agg_ps[:, st * TN:(st + 1) * TN],
                            lhsT=e_sup[:, q, :],
                            rhs=oh[:, q, :],
                            start=(c == 0),
                            stop=(c == CPB - 1),
                        )
                aggT = aggpool.tile([128, nn], MLP_DT, tag="aggT")
                nc.scalar.copy(aggT[:], agg_ps[:])
                # layer 1, hidden chunk by hidden chunk
                hT = hpool.tile([128, HC, nn], MLP_DT, tag="hT")
                for hc in range(HC):
                    h_ps = ps_h.tile([128, nn], f32, tag="h")
                    nc.tensor.matmul(h_ps[:],
                                     lhsT=w1a_sb[:, hc * 128:(hc + 1) * 128],
                                     rhs=aggT[:],
                                     start=True, stop=False)
                    nc.tensor.matmul(h_ps[:],
                                     lhsT=w1b_sb[:, hc * 128:(hc + 1) * 128],
                                     rhs=xT_sb[:, n0:n0 + nn],
                                     start=False, stop=True)
                    if hc % 2 == 0:
                        nc.scalar.activation(
                            hT[:, hc, :], h_ps[:],
                            mybir.ActivationFunctionType.Relu,
                            bias=b1T_sb[:, hc:hc + 1], scale=1.0)
                    else:
                        nc.vector.tensor_scalar(
                            out=hT[:, hc, :], in0=h_ps[:],
                            scalar1=b1T_sb[:, hc:hc + 1], scalar2=0.0,
                            op0=mybir.AluOpType.add,
                            op1=mybir.AluOpType.max)
                # layer 2: weights stationary, feature-major output
                o_ps = ps_out.tile([128, nn], f32, tag="ops")
                for hc in range(HC):
                    nc.tensor.matmul(
                        o_ps[:],
                        lhsT=w2r_sb[:, hc * 128:(hc + 1) * 128],
                        rhs=hT[:, hc, :],
                        start=(hc == 0), stop=(hc == HC - 1))
                o_sb = outpool.tile([128, nn], OUT_DT, tag="o")
                nc.scalar.activation(o_sb[:], o_ps[:],
                                     mybir.ActivationFunctionType.Identity,
                                     bias=b2T_sb[:, 0:1], scale=1.0)
                nc.scalar.dma_start(y[:, n0:n0 + nn], o_sb[:])
                t0 += nts

    nc.compile()
    nc.m = get_hw_module(nc.m)
    return nc


# ---------------- host-side sharding / packing ----------------

def _pack_bins(deg):
    """Snake-assign degree-sorted nodes into B_GLOBAL bins of <=32 nodes
    and (statistically) <=1024 edges.  Returns bin id + position-in-bin
    per node."""
    order = np.argsort(-deg, kind="stable")
    B = B_GLOBAL
    bsum = np.zeros(B, np.int64)
    bn = np.zeros(B, np.int64)
    bin_of = np.empty(N_NODES, np.int64)
    pos_of = np.empty(N_NODES, np.int64)
    n = len(order)
    for r in range((n + B - 1) // B):
        take = order[r * B:(r + 1) * B]
        ob = np.argsort(bsum, kind="stable")[:len(take)]
        bin_of[take] = ob
        pos_of[take] = bn[ob]
        bn[ob] += 1
        bsum[ob] += deg[take]
    # safety: if any bin exceeds the 1024-edge cap, move its smallest-
    # degree nodes to the emptiest bins with node room
    while True:
        over = np.flatnonzero(bsum > CPB * 128)
        if not len(over):
            break
        b = over[0]
        nodes = np.flatnonzero(bin_of == b)
        v = nodes[np.argmin(deg[nodes])]
        cand = np.flatnonzero(bn < TN)
        tgt = cand[np.argmin(bsum[cand])]
        if bsum[tgt] + deg[v] > CPB * 128:
            raise RuntimeError("bin packing failed")
        # re-compact positions in source bin
        pos_of[nodes[pos_of[nodes] > pos_of[v]]] -= 1
        bin_of[v] = tgt
        pos_of[v] = bn[tgt]
        bn[tgt] += 1
        bn[b] -= 1
        bsum[tgt] += deg[v]
        bsum[b] -= deg[v]
    return bin_of, pos_of


def prepare_inputs(x, edge_attr, u, W1, b1, W2, b2, edge_index):
    x = np.asarray(x, dtype=np.float32)
    edge_attr = np.asarray(edge_attr, dtype=np.float32)
    u = np.asarray(u, dtype=np.float32)
    W1 = np.asarray(W1, dtype=np.float32)
    b1 = np.asarray(b1, dtype=np.float32)
    W2 = np.asarray(W2, dtype=np.float32)
    b2 = np.asarray(b2, dtype=np.float32)
    recv = np.asarray(edge_index)[1].astype(np.int64)

    edge_np = _np(EDGE_DT)
    idx_np = _np(IDX_DT)
    mlp_np = _np(MLP_DT)

    deg = np.bincount(recv, minlength=N_NODES)
    bin_of, pos_of = _pack_bins(deg)

    # sort edges by (bin, pos-in-bin) of their receiver -> per-node runs
    node_key = bin_of * TN + pos_of            # globally unique per node
    ekey = node_key[recv]
    order = np.argsort(ekey, kind="stable")
    ekey_s = ekey[order]
    ea_s = edge_attr[order]

    # error-feedback fp8 quantization per receiver run: the device's
    # fp32 sum of q equals the true fp32 sum minus the final carry
    run_start = np.r_[True, ekey_s[1:] != ekey_s[:-1]]
    starts = np.flatnonzero(run_start)
    run_id = np.cumsum(run_start) - 1
    pos_in_run = np.arange(len(ekey_s)) - starts[run_id]
    q_s = np.empty((len(ekey_s), F), edge_np)
    carry = np.zeros((len(starts), F), np.float32)
    for p in range(int(pos_in_run.max()) + 1):
        sel = np.flatnonzero(pos_in_run == p)
        if not len(sel):
            break
        r = run_id[sel]
        v = ea_s[sel] + carry[r]
        qv = v.astype(edge_np)
        carry[r] = v - qv.astype(np.float32)
        q_s[sel] = qv

    # slot within bin: rank of edge inside its bin
    ebin = ekey_s // TN
    bin_start = np.searchsorted(ebin, np.arange(B_GLOBAL))
    rank = np.arange(len(ebin)) - bin_start[ebin]
    # chunk within core: bin (local) * CPB + rank // 128
    core_of = ebin // NT
    q_local = (ebin % NT) * CPB + (rank >> 7)
    p_slot = rank & 127
    flat = p_slot * QT + q_local               # within-core flat slot

    # shared (replicated) tensors
    b1_eff = b1 + (u[0] @ W1[256:256 + D_U])
    w1a = np.ascontiguousarray(W1[0:128]).astype(mlp_np)
    w1b = np.ascontiguousarray(W1[128:256]).astype(mlp_np)
    w2r = np.ascontiguousarray(
        W2.reshape(HC, 128, F).transpose(1, 0, 2).reshape(128, H)
    ).astype(mlp_np)
    b1T = np.ascontiguousarray(b1_eff.reshape(HC, 128).T).astype(np.float32)
    b2T = b2.reshape(128, 1).astype(np.float32)
    iota_arr = np.tile(np.arange(TN, dtype=np.float32),
                       (128, 1)).astype(idx_np)

    in_maps = []
    node_slot = np.empty(N_NODES, np.int64)  # per-core slot of each node
    for c in range(N_CORES):
        esel = core_of == c
        ebuf = np.zeros((128 * QT, F), edge_np)
        ebuf[flat[esel]] = q_s[esel]
        rel = np.full(128 * QT, -1.0, np.float32)
        rel[flat[esel]] = (ekey_s[esel] % TN).astype(np.float32)

        nsel = np.flatnonzero((bin_of >= c * NT) & (bin_of < (c + 1) * NT))
        slots = (bin_of[nsel] - c * NT) * TN + pos_of[nsel]
        node_slot[nsel] = slots
        xT_arr = np.zeros((128, NODES_PAD), mlp_np)
        xT_arr[:, slots] = x[nsel].T.astype(mlp_np)

        in_maps.append({
            "edges": ebuf.reshape(128, QT, F),
            "relT": rel.reshape(128, QT).astype(idx_np),
            "iota": iota_arr, "xT": xT_arr,
            "w1a": w1a, "w1b": w1b, "w2r": w2r, "b1T": b1T, "b2T": b2T,
        })
    core_of_node = bin_of // NT
    return in_maps, core_of_node, node_slot


_prog_cache = {}


def _get_program():
    key = (EDGE_DT, MLP_DT, OUT_DT)
    if key not in _prog_cache:
        _prog_cache[key] = build_program()
    return _prog_cache[key]


def run(inputs, trace=False, tmpdir=None):
    in_maps, core_of_node, node_slot = prepare_inputs(**inputs)
    nc = _get_program()
    res = bass_utils.run_bass_kernel_spmd(
        nc, in_maps, core_ids=list(range(N_CORES)), trace=trace,
        tmpdir=tmpdir)
    out = np.empty((N_NODES, F), np.float32)
    for c in range(N_CORES):
        yc = np.asarray(res.results[c]["y"], dtype=np.float32)  # [128, PAD]
        nsel = np.flatnonzero(core_of_node == c)
        out[nsel] = yc[:, node_slot[nsel]].T
    return out, res


def kernel(**inputs) -> np.ndarray:
    out, _ = run(inputs, trace=False)
    return out
